# revision 8
# baseline (speedup 1.0000x reference)
"""Trainium2 Bass kernel for the tied-weight Critic MLP.

Math (derived from the reference):
  x   = concat(inputs, actions)                  (B, 420), B = 8192
  s   = sum over 30 column-blocks of 14          (B, 14)
  y1  = s @ W1.T + b1                            (B, 512)
  h1  = relu(layernorm_512(y1))        [g1=1, beta1=0, LN over the 30x tile
                                        equals LN over one 512 block]
  y2  = h1 @ (30*W2).T + b2                      (B, 512)
  h2  = relu(layernorm_512(y2))
  V   = h2 @ (30*wV).T + bV                      (B, 1)
  out = tile(V, 30)                              (B, 30)

Sharding: pure data parallelism - batch 8192 split as 1024 rows on each of
8 NeuronCores; weights replicated.

Per-core kernel layout (batch-major, 8 tiles of 128 rows):
  s via strided free-axis reduce; s.T via PE transpose; mm1 with bias folded
  into an augmented ones-row (K=15); LN via bn_stats/bn_aggr + fused
  scale/bias Relu activation; h1.T via 4 PE transposes into one PSUM bank;
  mm2 = b2-broadcast matmul (K=1) + 4 accumulating K=128 matmuls; V via
  fused tensor_tensor_reduce against broadcast wV; output broadcast to 30
  columns with a per-partition-scaled activation.
"""

import numpy as np

N_CORES = 8
B_FULL = 8192
B_CORE = B_FULL // N_CORES  # 1024
P = 128
N_TILES = B_CORE // P  # 8
N_AGENTS = 30
IN_F = 14
HID = 512
EPS = 1e-5

_cache = {}


def _build(bV: float, loop_n: int = 1):
    import concourse.bass as bass
    import concourse.tile as tile
    from concourse import bacc, mybir
    from concourse.bass import ts
    from concourse.masks import make_identity

    f32 = mybir.dt.float32
    AF = mybir.ActivationFunctionType
    ALU = mybir.AluOpType

    nc = bacc.Bacc("TRN2")

    xin_d = nc.dram_tensor("xin", (B_CORE, 360), f32, kind="ExternalInput")
    xact_d = nc.dram_tensor("xact", (B_CORE, 60), f32, kind="ExternalInput")
    w1t_d = nc.dram_tensor("w1t", (IN_F + 1, HID), f32, kind="ExternalInput")
    w2t_d = nc.dram_tensor("w2t", (HID, HID), f32, kind="ExternalInput")
    b2r_d = nc.dram_tensor("b2r", (1, HID), f32, kind="ExternalInput")
    wvr_d = nc.dram_tensor("wvr", (1, HID), f32, kind="ExternalInput")
    out_d = nc.dram_tensor("out", (B_CORE, N_AGENTS), f32, kind="ExternalOutput")

    with tile.TileContext(nc) as tc:
        with (
            tc.tile_pool(name="singles", bufs=1) as singles,
            tc.tile_pool(name="xp", bufs=3) as xp,
            tc.tile_pool(name="sp", bufs=3) as sp,
            tc.tile_pool(name="hp", bufs=2) as hp,
            tc.tile_pool(name="stat", bufs=6) as stat,
            tc.tile_pool(name="op", bufs=3) as op,
            tc.tile_pool(name="ps_y", bufs=2, space="PSUM") as ps_y,
            tc.tile_pool(name="ps_t", bufs=2, space="PSUM") as ps_t,
        ):
            # ---- constants / replicated weights ----
            ident = singles.tile([P, P], f32)
            make_identity(nc, ident)
            ones1 = singles.tile([1, P], f32)
            nc.vector.memset(ones1, 1.0)
            ones30 = singles.tile([P, N_AGENTS], f32)
            nc.vector.memset(ones30, 1.0)
            eps_t = singles.tile([P, 1], f32)
            nc.vector.memset(eps_t, EPS)

            w1t = singles.tile([IN_F + 1, HID], f32)
            nc.sync.dma_start(out=w1t, in_=w1t_d[:, :])
            w2sb = singles.tile([P, 4, HID], f32)
            nc.sync.dma_start(
                out=w2sb, in_=w2t_d[:, :].rearrange("(c p) n -> p c n", p=P)
            )
            b2r = singles.tile([1, HID], f32)
            nc.sync.dma_start(out=b2r, in_=b2r_d[:, :])
            # broadcast 30*wV to all 128 partitions
            wv_bc = singles.tile([P, HID], f32)
            wv_ap = wvr_d[:, :]
            nc.gpsimd.dma_start(
                out=wv_bc,
                in_=bass.AP(
                    tensor=wv_ap.tensor, offset=wv_ap.offset, ap=[[0, P]] + wv_ap.ap[1:]
                ),
            )

            def layer_norm_relu(y_psum, h_out):
                st6 = stat.tile([P, 6], f32, tag="st6")
                nc.vector.bn_stats(st6, y_psum)
                mv = stat.tile([P, 2], f32, tag="mv")
                nc.vector.bn_aggr(mv, st6)
                rstd = stat.tile([P, 1], f32, tag="rstd")
                nc.scalar.activation(rstd, mv[:, 1:2], AF.Sqrt, bias=eps_t, scale=1.0)
                nc.vector.reciprocal(rstd, rstd)
                nm = stat.tile([P, 1], f32, tag="nm")
                # nm = -mean * rstd
                nc.vector.tensor_scalar(
                    out=nm,
                    in0=mv[:, 0:1],
                    scalar1=-1.0,
                    scalar2=None,
                    op0=ALU.mult,
                )
                nc.vector.tensor_mul(nm, nm, rstd)
                # h = relu(y * rstd + nm) = relu((y - mean) * rstd)
                nc.scalar.activation(h_out, y_psum, AF.Relu, bias=nm, scale=rstd)

            def tile_body(i):
                rows = slice(i * P, (i + 1) * P)
                x_t = xp.tile([P, 360 + 60], f32)
                nc.sync.dma_start(out=x_t[:, 0:360], in_=xin_d[rows, :])
                nc.sync.dma_start(out=x_t[:, 360:420], in_=xact_d[rows, :])

                # s[b, f] = sum_a x[b, 14a + f]
                s_t = sp.tile([P, IN_F], f32, tag="s")
                nc.vector.reduce_sum(
                    s_t,
                    x_t[:, :].rearrange("p (a f) -> p f a", f=IN_F),
                    axis=mybir.AxisListType.X,
                )
                # s.T augmented with a ones row (bias fold)
                stp = ps_t.tile([IN_F, P], f32, tag="stp")
                nc.tensor.transpose(stp, s_t, ident)
                st_sb = sp.tile([IN_F + 1, P], f32, tag="st")
                nc.vector.memset(st_sb, 1.0)
                nc.vector.tensor_copy(st_sb[0:IN_F, :], stp)

                # y1 = [s | 1] @ [W1 | b1].T
                y1 = ps_y.tile([P, HID], f32, tag="y1")
                nc.tensor.matmul(y1, st_sb, w1t, start=True, stop=True)

                h1 = hp.tile([P, HID], f32, tag="h1")
                layer_norm_relu(y1, h1)

                # h1.T chunks -> one PSUM bank, then one copy to SBUF
                tp = ps_t.tile([P, HID], f32, tag="tp")
                for j in range(4):
                    nc.tensor.transpose(tp[:, ts(j, P)], h1[:, ts(j, P)], ident)
                h1t = hp.tile([P, HID], f32, tag="h1t")
                nc.vector.tensor_copy(h1t, tp)

                # y2 = b2 + sum_j h1t_j.T @ w2t_j   (w2 prescaled by 30)
                y2 = ps_y.tile([P, HID], f32, tag="y2")
                nc.tensor.matmul(y2, ones1, b2r, start=True, stop=False)
                for j in range(4):
                    nc.tensor.matmul(
                        y2,
                        h1t[:, ts(j, P)],
                        w2sb[:, j, :],
                        start=False,
                        stop=(j == 3),
                    )

                h2 = hp.tile([P, HID], f32, tag="h2")
                layer_norm_relu(y2, h2)

                # V = sum_f h2 * (30*wV)  (fused mul+reduce), out = V*1 + bV
                tmp = hp.tile([P, HID], f32, tag="tmp")
                v_t = stat.tile([P, 1], f32, tag="v")
                nc.vector.tensor_mul(tmp, h2, wv_bc)
                nc.vector.reduce_sum(v_t, tmp, axis=mybir.AxisListType.X)
                o30 = op.tile([P, N_AGENTS], f32, tag="o30")
                nc.scalar.activation(o30, ones30, AF.Copy, bias=float(bV), scale=v_t)
                nc.sync.dma_start(out=out_d[rows, :], in_=o30)

            def body():
                for i in range(N_TILES):
                    tile_body(i)

            if loop_n > 1:
                # timing amplification: repeat the identical batch loop_n times
                with tc.For_i(0, loop_n, 1):
                    body()
            else:
                body()

    nc.compile()
    return nc


def _prep(inputs):
    xin = np.ascontiguousarray(inputs["inputs"], dtype=np.float32)
    xact = np.ascontiguousarray(inputs["actions"], dtype=np.float32)
    w1 = np.asarray(inputs["w1"], np.float32)
    b1 = np.asarray(inputs["b1"], np.float32)
    w2 = np.asarray(inputs["w2"], np.float32)
    b2 = np.asarray(inputs["b2"], np.float32)
    wV = np.asarray(inputs["wV"], np.float32)
    bV = float(np.asarray(inputs["bV"], np.float32).reshape(-1)[0])

    # LN affine params are identity in this model; the kernel folds them away.
    for k, want in (("g1", 1.0), ("g2", 1.0), ("beta1", 0.0), ("beta2", 0.0)):
        if k in inputs:
            assert np.allclose(np.asarray(inputs[k]), want), f"{k} must be {want}"

    w1t = np.ascontiguousarray(
        np.concatenate([w1, b1[:, None]], axis=1).T, dtype=np.float32
    )  # (15, 512)
    w2t = np.ascontiguousarray((N_AGENTS * w2).T, dtype=np.float32)  # (512, 512)
    b2r = np.ascontiguousarray(b2[None, :], dtype=np.float32)  # (1, 512)
    wvr = np.ascontiguousarray(N_AGENTS * wV.reshape(1, -1), dtype=np.float32)

    in_maps = []
    for c in range(N_CORES):
        rows = slice(c * B_CORE, (c + 1) * B_CORE)
        in_maps.append(
            {
                "xin": xin[rows],
                "xact": xact[rows],
                "w1t": w1t,
                "w2t": w2t,
                "b2r": b2r,
                "wvr": wvr,
            }
        )
    return in_maps, bV


def _run(inputs, trace=False):
    from concourse.bass_utils import run_bass_kernel_spmd

    in_maps, bV = _prep(inputs)
    if "nc" not in _cache:
        _cache["nc"] = _build(bV)
    res = run_bass_kernel_spmd(
        _cache["nc"], in_maps, core_ids=list(range(N_CORES)), trace=trace
    )
    out = np.concatenate([m["out"] for m in res.results], axis=0)
    return out, res


def kernel(**inputs) -> np.ndarray:
    out, _ = _run(inputs, trace=False)
    return out


# revision 11
# speedup vs baseline: 2.6271x; 2.6271x over previous
"""Trainium2 Bass kernel for the tied-weight Critic MLP.

Math (derived from the reference):
  x   = concat(inputs, actions)                  (B, 420), B = 8192
  s   = sum over 30 column-blocks of 14          (B, 14)
  y1  = s @ W1.T + b1                            (B, 512)
  h1  = relu(layernorm_512(y1))        [g1=1, beta1=0, LN over the 30x tile
                                        equals LN over one 512 block]
  y2  = h1 @ (30*W2).T + b2                      (B, 512)
  h2  = relu(layernorm_512(y2))
  V   = h2 @ (30*wV).T + bV                      (B, 1)
  out = tile(V, 30)                              (B, 30)

Sharding: pure data parallelism - batch 8192 split as 1024 rows on each of
8 NeuronCores; weights replicated.

Per-core kernel layout (batch-major, 8 tiles of 128 rows, processed in
pairs):
  s via strided free-axis reduce; s.T for two tiles via one PE transpose
  (tiles placed 32 partitions apart so both lhsT slices stay 32-aligned);
  mm1 with bias folded into an augmented ones-row (K=15); LayerNorm via
  bn_stats/bn_aggr with a fused scale/bias ReLU activation that emits h1
  in bf16; h1.T via DMA-engine transposes (keeps the PE free); mm2 as 4
  accumulating bf16 matmuls with b2 added on the vector engine from a
  broadcast tile; V via mul+reduce against broadcast wV; output broadcast
  to 30 columns with a per-partition-scaled activation.
"""

import numpy as np

N_CORES = 8
B_FULL = 8192
B_CORE = B_FULL // N_CORES  # 1024
P = 128
N_TILES = B_CORE // P  # 8
N_AGENTS = 30
IN_F = 14
HID = 512
EPS = 1e-5

_cache = {}


def _build(bV: float, loop_n: int = 1):
    import concourse.bass as bass
    import concourse.tile as tile
    from concourse import bacc, mybir
    from concourse.bass import ts
    from concourse.masks import make_identity

    f32 = mybir.dt.float32
    bf16 = mybir.dt.bfloat16
    AF = mybir.ActivationFunctionType
    ALU = mybir.AluOpType

    nc = bacc.Bacc("TRN2")

    xin_d = nc.dram_tensor("xin", (B_CORE, 360), f32, kind="ExternalInput")
    xact_d = nc.dram_tensor("xact", (B_CORE, 60), f32, kind="ExternalInput")
    w1t_d = nc.dram_tensor("w1t", (IN_F + 1, HID), f32, kind="ExternalInput")
    w2t_d = nc.dram_tensor("w2t", (HID, HID), bf16, kind="ExternalInput")
    b2r_d = nc.dram_tensor("b2r", (1, HID), f32, kind="ExternalInput")
    wvr_d = nc.dram_tensor("wvr", (1, HID), f32, kind="ExternalInput")
    out_d = nc.dram_tensor("out", (B_CORE, N_AGENTS), f32, kind="ExternalOutput")

    def bcast(ap, p=P):
        return bass.AP(tensor=ap.tensor, offset=ap.offset, ap=[[0, p]] + ap.ap[1:])

    with tile.TileContext(nc) as tc:
        with (
            tc.tile_pool(name="singles", bufs=1) as singles,
            tc.tile_pool(name="xp", bufs=4) as xp,
            tc.tile_pool(name="sp", bufs=2) as sp,
            tc.tile_pool(name="hp", bufs=3) as hp,
            tc.tile_pool(name="stat", bufs=8) as stat,
            tc.tile_pool(name="op", bufs=3) as op,
            tc.tile_pool(name="ps_y", bufs=3, space="PSUM") as ps_y,
            tc.tile_pool(name="ps_t", bufs=2, space="PSUM") as ps_t,
        ):
            # ---- constants / replicated weights ----
            ident = singles.tile([P, P], f32)
            make_identity(nc, ident)
            ones30 = singles.tile([P, N_AGENTS], f32)
            nc.vector.memset(ones30, 1.0)
            eps_t = singles.tile([P, 1], f32)
            nc.vector.memset(eps_t, EPS)

            # w1t replicated at partitions 0 and 32 (matmul requires lhsT and
            # rhs to share a base partition; pair member a=1 sits at 32)
            w1t = singles.tile([32 + IN_F + 1, HID], f32)
            nc.sync.dma_start(out=w1t[0 : IN_F + 1, :], in_=w1t_d[:, :])
            nc.sync.dma_start(out=w1t[32 : 32 + IN_F + 1, :], in_=w1t_d[:, :])
            w2sb = singles.tile([P, 4, HID], bf16)
            nc.sync.dma_start(
                out=w2sb, in_=w2t_d[:, :].rearrange("(c p) n -> p c n", p=P)
            )
            b2_bc = singles.tile([P, HID], f32)
            nc.gpsimd.dma_start(out=b2_bc, in_=bcast(b2r_d[:, :]))
            wv_bc = singles.tile([P, HID], f32)
            nc.gpsimd.dma_start(out=wv_bc, in_=bcast(wvr_d[:, :]))

            def layer_norm_relu(y_in, h_out):
                st6 = stat.tile([P, 6], f32, tag="st6")
                nc.vector.bn_stats(st6, y_in)
                mv = stat.tile([P, 2], f32, tag="mv")
                nc.vector.bn_aggr(mv, st6)
                rstd = stat.tile([P, 1], f32, tag="rstd")
                nc.scalar.activation(rstd, mv[:, 1:2], AF.Sqrt, bias=eps_t, scale=1.0)
                nc.vector.reciprocal(rstd, rstd)
                nm = stat.tile([P, 1], f32, tag="nm")
                # nm = -mean * rstd
                nc.vector.tensor_scalar(
                    out=nm, in0=mv[:, 0:1], scalar1=-1.0, scalar2=None, op0=ALU.mult
                )
                nc.vector.tensor_mul(nm, nm, rstd)
                # h = relu(y * rstd + nm) = relu((y - mean) * rstd)
                nc.scalar.activation(h_out, y_in, AF.Relu, bias=nm, scale=rstd)

            def pair_body(pi):
                # ---- load x and form s for both tiles of the pair ----
                x_ts = []
                for a in (0, 1):
                    rows = slice((2 * pi + a) * P, (2 * pi + a + 1) * P)
                    x_t = xp.tile([P, 420], f32, tag="x")
                    nc.sync.dma_start(out=x_t[:, 0:360], in_=xin_d[rows, :])
                    nc.sync.dma_start(out=x_t[:, 360:420], in_=xact_d[rows, :])
                    x_ts.append(x_t)

                # two s blocks 32 partitions apart so both transposed lhsT
                # slices are 32-aligned
                s2 = sp.tile([P, 64], f32, tag="s2")
                nc.vector.memset(s2, 0.0)
                for a in (0, 1):
                    nc.vector.reduce_sum(
                        s2[:, 32 * a : 32 * a + IN_F],
                        x_ts[a][:, :].rearrange("p (a f) -> p f a", f=IN_F),
                        axis=mybir.AxisListType.X,
                    )
                stp2 = ps_t.tile([64, P], f32, tag="stp2")
                nc.tensor.transpose(stp2, s2, ident)
                st2 = sp.tile([64, P], f32, tag="st2")
                nc.vector.memset(st2, 1.0)
                for a in (0, 1):
                    nc.vector.tensor_copy(
                        st2[32 * a : 32 * a + IN_F, :], stp2[32 * a : 32 * a + IN_F, :]
                    )

                for a in (0, 1):
                    rows = slice((2 * pi + a) * P, (2 * pi + a + 1) * P)
                    # ---- y1 = [s | 1] @ [W1 | b1].T ----
                    y1 = ps_y.tile([P, HID], f32, tag="y1")
                    nc.tensor.matmul(
                        y1,
                        st2[32 * a : 32 * a + IN_F + 1, :],
                        w1t[32 * a : 32 * a + IN_F + 1, :],
                        start=True,
                        stop=True,
                    )

                    h1b = hp.tile([P, HID], bf16, tag="h1b")
                    layer_norm_relu(y1, h1b)

                    # ---- h1.T via DMA transposes (bf16) ----
                    h1t = hp.tile([P, 4, P], bf16, tag="h1t")
                    for j in range(4):
                        nc.sync.dma_start_transpose(h1t[:, j, :], h1b[:, ts(j, P)])

                    # ---- y2 = 30*h1 @ W2.T, then + b2 on DVE ----
                    y2 = ps_y.tile([P, HID], f32, tag="y2")
                    for j in range(4):
                        nc.tensor.matmul(
                            y2,
                            h1t[:, j, :],
                            w2sb[:, j, :],
                            start=(j == 0),
                            stop=(j == 3),
                        )
                    y2b = hp.tile([P, HID], f32, tag="y2b")
                    nc.vector.tensor_add(y2b, y2, b2_bc)

                    h2 = hp.tile([P, HID], f32, tag="h2")
                    layer_norm_relu(y2b, h2)

                    # ---- V = sum_f h2 * (30*wV); out = V + bV ----
                    tmp = hp.tile([P, HID], f32, tag="tmp")
                    v_t = stat.tile([P, 1], f32, tag="v")
                    nc.vector.tensor_mul(tmp, h2, wv_bc)
                    nc.vector.reduce_sum(v_t, tmp, axis=mybir.AxisListType.X)
                    o30 = op.tile([P, N_AGENTS], f32, tag="o30")
                    nc.scalar.activation(
                        o30, ones30, AF.Copy, bias=float(bV), scale=v_t
                    )
                    nc.sync.dma_start(out=out_d[rows, :], in_=o30)

            def body():
                for pi in range(N_TILES // 2):
                    pair_body(pi)

            if loop_n > 1:
                # timing amplification: repeat the identical batch loop_n times
                with tc.For_i(0, loop_n, 1):
                    body()
            else:
                body()

    nc.compile()
    return nc


def _prep(inputs):
    import ml_dtypes

    xin = np.ascontiguousarray(inputs["inputs"], dtype=np.float32)
    xact = np.ascontiguousarray(inputs["actions"], dtype=np.float32)
    w1 = np.asarray(inputs["w1"], np.float32)
    b1 = np.asarray(inputs["b1"], np.float32)
    w2 = np.asarray(inputs["w2"], np.float32)
    b2 = np.asarray(inputs["b2"], np.float32)
    wV = np.asarray(inputs["wV"], np.float32)
    bV = float(np.asarray(inputs["bV"], np.float32).reshape(-1)[0])

    # LN affine params are identity in this model; the kernel folds them away.
    for k, want in (("g1", 1.0), ("g2", 1.0), ("beta1", 0.0), ("beta2", 0.0)):
        if k in inputs:
            assert np.allclose(np.asarray(inputs[k]), want), f"{k} must be {want}"

    w1t = np.ascontiguousarray(
        np.concatenate([w1, b1[:, None]], axis=1).T, dtype=np.float32
    )  # (15, 512)
    w2t = np.ascontiguousarray((N_AGENTS * w2).T).astype(ml_dtypes.bfloat16)
    b2r = np.ascontiguousarray(b2[None, :], dtype=np.float32)  # (1, 512)
    wvr = np.ascontiguousarray(N_AGENTS * wV.reshape(1, -1), dtype=np.float32)

    in_maps = []
    for c in range(N_CORES):
        rows = slice(c * B_CORE, (c + 1) * B_CORE)
        in_maps.append(
            {
                "xin": xin[rows],
                "xact": xact[rows],
                "w1t": w1t,
                "w2t": w2t,
                "b2r": b2r,
                "wvr": wvr,
            }
        )
    return in_maps, bV


def _run(inputs, trace=False):
    from concourse.bass_utils import run_bass_kernel_spmd

    in_maps, bV = _prep(inputs)
    if "nc" not in _cache:
        _cache["nc"] = _build(bV)
    res = run_bass_kernel_spmd(
        _cache["nc"], in_maps, core_ids=list(range(N_CORES)), trace=trace
    )
    out = np.concatenate([m["out"] for m in res.results], axis=0)
    return out, res


def kernel(**inputs) -> np.ndarray:
    out, _ = _run(inputs, trace=False)
    return out
